# revision 26
# baseline (speedup 1.0000x reference)
"""Trainium2 Bass kernel for PhysicsInformedMHDSolver.

Data-parallel over 8 NeuronCores: each core runs batch shard of 2048 rows
through MLP (8->256->512->256->4096, gelu+LN via erf + folded affine),
tanh -> segmented cummax (single DVE scan w/ additive reset mask) ->
sigmoid (+row-sum accum) -> 5-pt Laplacian residual accum.
Host combines per-row partials into pressure / physics_loss and builds the
constant q_profile.
"""

import numpy as np
import ml_dtypes

BF16 = ml_dtypes.bfloat16
GRID = 64
NG = GRID * GRID          # 4096
B = 16384
NCORES = 8
BS = B // NCORES          # 2048 per core
P = 128
NT = BS // P              # 16 tiles per core
SQ2INV = 0.7071067811865476
EPS = 1e-5
HID = [256, 512, 256]

_CACHE = {}

# results of the last device run (test.py reads this for profiling)
LAST_RESULTS = None


def _build_bass():
    import concourse.bacc as bacc
    import concourse.mybir as mybir
    import concourse.tile as tile
    from contextlib import ExitStack

    dt = mybir.dt
    f32 = dt.float32
    u32 = dt.uint32
    bf16 = dt.bfloat16
    Alu = mybir.AluOpType
    Act = mybir.ActivationFunctionType

    nc = bacc.Bacc()

    # ---- DRAM I/O ----
    pT = nc.dram_tensor("pT", [9, BS], bf16, kind="ExternalInput")       # plasma^T + ones row
    negs = nc.dram_tensor("negs", [BS, 1], f32, kind="ExternalInput")   # -plasma[:,0]
    we = nc.dram_tensor("we", [9, HID[0]], bf16, kind="ExternalInput")   # [W_enc; b_enc]
    w1 = nc.dram_tensor("w1", [HID[0], HID[1]], bf16, kind="ExternalInput")
    a1w = nc.dram_tensor("a1w", [1, HID[1]], bf16, kind="ExternalInput")  # [bias']
    w2 = nc.dram_tensor("w2", [HID[1], HID[2]], bf16, kind="ExternalInput")
    a2w = nc.dram_tensor("a2w", [1, HID[2]], bf16, kind="ExternalInput")
    wf = nc.dram_tensor("wf", [HID[2], NG], bf16, kind="ExternalInput")
    afw = nc.dram_tensor("afw", [1, NG], bf16, kind="ExternalInput")
    ident = nc.dram_tensor("ident", [P, P], bf16, kind="ExternalInput")
    mask = nc.dram_tensor("mask", [P, NG], f32, kind="ExternalInput")   # scan reset mask
    oflux = nc.dram_tensor("oflux", [BS, NG], f32, kind="ExternalOutput")
    ofsum = nc.dram_tensor("ofsum", [P, NT], f32, kind="ExternalOutput")
    ogs = nc.dram_tensor("ogs", [P, NT], f32, kind="ExternalOutput")

    with ExitStack() as ctx:
        tc = ctx.enter_context(tile.TileContext(nc))
        const = ctx.enter_context(tc.tile_pool(name="const", bufs=1))
        pst = ctx.enter_context(tc.tile_pool(name="pst", bufs=6))
        pwork = ctx.enter_context(tc.tile_pool(name="pwork", bufs=3))
        pbig = ctx.enter_context(tc.tile_pool(name="pbig", bufs=2))
        pfl = ctx.enter_context(tc.tile_pool(name="pfl", bufs=2))
        plap = ctx.enter_context(tc.tile_pool(name="plap", bufs=2))
        ppz = ctx.enter_context(tc.tile_pool(name="ppz", bufs=3, space="PSUM"))
        ppt = ctx.enter_context(tc.tile_pool(name="ppt", bufs=1, space="PSUM"))
        ppf = ctx.enter_context(tc.tile_pool(name="ppf", bufs=2, space="PSUM"))

        # ---- load constants into SBUF ----
        we_sb = const.tile([9, HID[0]], bf16)
        nc.sync.dma_start(we_sb[:], we[:])
        w1_sb = []
        for j in range(2):
            w1j = const.tile([P, HID[1]], bf16, tag=f"w1_{j}", name=f"w1_{j}")
            nc.sync.dma_start(w1j[:], w1[j * P:(j + 1) * P, :])
            w1_sb.append(w1j)
        a1w_sb = const.tile([1, HID[1]], bf16)
        nc.sync.dma_start(a1w_sb[:], a1w[:])
        w2_sb = []
        for j in range(4):
            w2j = const.tile([P, HID[2]], bf16, tag=f"w2_{j}", name=f"w2_{j}")
            nc.sync.dma_start(w2j[:], w2[j * P:(j + 1) * P, :])
            w2_sb.append(w2j)
        a2w_sb = const.tile([1, HID[2]], bf16)
        nc.sync.dma_start(a2w_sb[:], a2w[:])
        wf_sb = []
        for j in range(2):
            wfj = const.tile([P, NG], bf16, tag=f"wf_{j}", name=f"wf_{j}")
            nc.sync.dma_start(wfj[:], wf[j * P:(j + 1) * P, :])
            wf_sb.append(wfj)
        afw_sb = const.tile([1, NG], bf16)
        nc.sync.dma_start(afw_sb[:], afw[:])
        ident_sb = const.tile([P, P], bf16)
        nc.sync.dma_start(ident_sb[:], ident[:])
        mask_sb = const.tile([P, NG], f32)
        nc.sync.dma_start(mask_sb[:], mask[:])

        one_i = const.tile([P, 1], u32)
        nc.vector.memset(one_i[:], 1)
        magic_i = const.tile([P, 1], u32)
        nc.vector.memset(magic_i[:], 0x5F3759DF)

        ones_row = const.tile([1, P], bf16)
        nc.vector.memset(ones_row[:], 1.0)
        fsum_acc = const.tile([P, NT], f32)
        gs_acc = const.tile([P, NT], f32)

        # PE warm-reads of every weight tile: makes the tensor engine's
        # vector clock observe all const-DMA queue semaphores once, so the
        # real matmul groups don't exceed the per-instruction sync-wait cap.
        pe_read = [we_sb, *w1_sb, a1w_sb, *w2_sb, a2w_sb, *wf_sb, afw_sb,
                   ident_sb]
        for i, cts in enumerate(pe_read):
            dz = ppt.tile([1, 1], f32, tag="tp", name=f"dz{i}")
            nc.tensor.matmul(dz[:], lhsT=cts[0:1, 0:1], rhs=cts[0:1, 0:1],
                             start=True, stop=True)

        def gelu_norm(z, n, lname):
            """z: PSUM [P,n] true pre-gelu.  Returns un = LayerNorm(gelu(z))
            (unit affine; g/beta folded into weights host-side).  Uses
            u = (1+erf(z/sqrt2))*z = 2*gelu(z); LN is scale-invariant up to
            the eps term, handled exactly via ve = var(u) + 4*eps."""
            e = pwork.tile([P, n], bf16, tag=f"e{lname}", name=f"e{lname}")
            nc.scalar.activation(e[:], z[:], Act.Erf, bias=0.0, scale=SQ2INV)
            u = pwork.tile([P, n], bf16, tag=f"u{lname}", name=f"u{lname}")
            nc.vector.scalar_tensor_tensor(u[:], e[:], 1.0, z[:], Alu.add, Alu.mult)
            st = pst.tile([P, 6], f32, tag="st", name="st")
            nc.vector.bn_stats(st[:], u[:])
            mv = pst.tile([P, 2], f32, tag="mv", name="mv")
            nc.vector.bn_aggr(mv[:], st[:])
            mu = mv[:, 0:1]
            ve = pst.tile([P, 1], f32, tag="ve", name="ve")
            nc.vector.tensor_scalar_add(ve[:], mv[:, 1:2], 4.0 * EPS)
            # rsqrt via magic-constant + Newton (all DVE; no ACT table swap)
            ish = pst.tile([P, 1], u32, tag="ish", name="ish")
            nc.vector.tensor_tensor(ish[:], ve[:].bitcast(u32), one_i[:],
                                    Alu.logical_shift_right)
            y = pst.tile([P, 1], f32, tag=f"y{lname}", name=f"y{lname}")
            nc.vector.tensor_tensor(y[:].bitcast(u32), magic_i[:], ish[:], Alu.subtract)
            tn = pst.tile([P, 1], f32, tag="tn", name="tn")
            for _ in range(3):
                nc.vector.tensor_mul(tn[:], y[:], y[:])
                nc.vector.tensor_mul(tn[:], tn[:], ve[:])
                nc.vector.tensor_scalar(tn[:], tn[:], -0.5, 1.5, Alu.mult, Alu.add)
                nc.vector.tensor_mul(y[:], y[:], tn[:])
            un = pwork.tile([P, n], bf16, tag=f"un{lname}", name=f"un{lname}")
            nc.vector.tensor_scalar(un[:], u[:], mu, y[:],
                                    Alu.subtract, Alu.mult)
            return un

        def transpose_set(u, nblk, lname, use_pe):
            uT = pwork.tile([P, nblk * P], bf16, tag=f"uT{lname}", name=f"uT{lname}")
            for j in range(nblk):
                if use_pe:
                    tp = ppt.tile([P, P], bf16, tag="tp", name="tp")
                    nc.tensor.transpose(tp[:], u[:, j * P:(j + 1) * P], ident_sb[:])
                    nc.vector.tensor_copy(uT[:, j * P:(j + 1) * P], tp[:])
                else:
                    nc.sync.dma_start(uT[:, j * P:(j + 1) * P],
                                      u[:, j * P:(j + 1) * P], transpose=True)
            return uT

        def do_mlp(t):
            # ---- encoder ----
            xT = pwork.tile([9, P], bf16, tag="xT", name="xT")
            nc.sync.dma_start(xT[:], pT[:, t * P:(t + 1) * P])
            z1 = ppz.tile([P, HID[0]], f32, tag="z", name="z1")
            nc.tensor.matmul(z1[:], lhsT=xT[:], rhs=we_sb[:], start=True, stop=True)
            u1 = gelu_norm(z1, HID[0], "1")

            # ---- layer 1: 256 -> 512 ----
            u1T = transpose_set(u1, 2, "1", use_pe=True)
            z2 = ppz.tile([P, HID[1]], f32, tag="z", name="z2")
            for j in range(2):
                nc.tensor.matmul(z2[:], lhsT=u1T[:, j * P:(j + 1) * P],
                                 rhs=w1_sb[j][:], start=(j == 0), stop=False)
            nc.tensor.matmul(z2[:], lhsT=ones_row[:], rhs=a1w_sb[:], start=False, stop=True)
            u2 = gelu_norm(z2, HID[1], "2")

            # ---- layer 2: 512 -> 256 ----
            u2T = transpose_set(u2, 4, "2", use_pe=True)
            z3 = ppz.tile([P, HID[2]], f32, tag="z", name="z3")
            for j in range(4):
                nc.tensor.matmul(z3[:], lhsT=u2T[:, j * P:(j + 1) * P],
                                 rhs=w2_sb[j][:], start=(j == 0), stop=False)
            nc.tensor.matmul(z3[:], lhsT=ones_row[:], rhs=a2w_sb[:], start=False, stop=True)
            u3 = gelu_norm(z3, HID[2], "3")

            # ---- final layer: 256 -> 4096, tanh ----
            u3T = transpose_set(u3, 2, "3", use_pe=True)
            return u3T

        def stage_b(t, st):
            u3T = st
            fy = pbig.tile([P, NG], f32, tag="fbig", name="fy")
            for q in range(4):
                pf = ppf.tile([P, 1024], f32, tag="pf", name="pf")
                for j in range(2):
                    for s in range(2):
                        off = q * 1024 + s * 512
                        nc.tensor.matmul(pf[:, s * 512:(s + 1) * 512],
                                         lhsT=u3T[:, j * P:(j + 1) * P],
                                         rhs=wf_sb[j][:, off:off + 512],
                                         start=(j == 0), stop=False)
                for s in range(2):
                    off = q * 1024 + s * 512
                    nc.tensor.matmul(pf[:, s * 512:(s + 1) * 512], lhsT=ones_row[:],
                                     rhs=afw_sb[:, off:off + 512],
                                     start=False, stop=True)
                nc.scalar.activation(fy[:, q * 1024:(q + 1) * 1024], pf[:],
                                     Act.Tanh, bias=0.0, scale=1.0)
            # cummax along each 64-wide row: one segmented scan
            fx = pbig.tile([P, NG], f32, tag="fbig", name="fx")
            nc.vector.tensor_tensor_scan(fx[:], mask_sb[:], fy[:], 0.0,
                                         Alu.add, Alu.max)
            return fx

        def stage_c(t, fx):
            fl = pfl.tile([P, NG], f32, tag="fl", name="fl")
            nc.scalar.activation(fl[:], fx[:], Act.Sigmoid, bias=0.0, scale=1.0,
                                 accum_out=fsum_acc[:, t:t + 1])
            nc.sync.dma_start(oflux[t * P:(t + 1) * P, :], fl[:])
            return fl

        def stage_cc(t, fl):
            return fl

        def stage_d(t, fl):
            f3 = fl[:].rearrange("p (r c) -> p r c", r=GRID, c=GRID)
            a1t = plap.tile([P, 62, 62], f32, tag="lapA", name="a1t")
            nc.vector.tensor_add(a1t[:], f3[:, 0:62, 1:63], f3[:, 2:64, 1:63])
            a2t = plap.tile([P, 62, 62], f32, tag="lapB", name="a2t")
            nc.vector.tensor_add(a2t[:], f3[:, 1:63, 0:62], f3[:, 1:63, 2:64])
            nc.vector.scalar_tensor_tensor(a1t[:], f3[:, 1:63, 1:63], -4.0, a1t[:],
                                           Alu.mult, Alu.add)
            nc.vector.tensor_add(a1t[:], a1t[:], a2t[:])
            return a1t, a2t

        def stage_e(t, lap):
            a1t, a2t = lap
            ns = pst.tile([P, 1], f32, tag="ns", name="ns")
            nc.sync.dma_start(ns[:], negs[t * P:(t + 1) * P, :])
            nc.scalar.activation(a2t[:], a1t[:], Act.Square, bias=ns[:], scale=1.0,
                                 accum_out=gs_acc[:, t:t + 1])

        # 5-deep software pipeline: each engine's in-order queue always has
        # ready work; the long scan->sigmoid->lap tail of tile t overlaps the
        # MLP/matmul stages of tiles t+1..t+4.
        sa, sb, sc, scc, sd = {}, {}, {}, {}, {}
        for t in range(NT + 5):
            if t < NT:
                sa[t] = do_mlp(t)
            if 0 <= t - 1 < NT:
                sb[t - 1] = stage_b(t - 1, sa.pop(t - 1))
            if 0 <= t - 2 < NT:
                sc[t - 2] = stage_c(t - 2, sb.pop(t - 2))
            if 0 <= t - 3 < NT:
                scc[t - 3] = stage_cc(t - 3, sc.pop(t - 3))
            if 0 <= t - 4 < NT:
                sd[t - 4] = stage_d(t - 4, scc.pop(t - 4))
            if 0 <= t - 5 < NT:
                stage_e(t - 5, sd.pop(t - 5))

        nc.sync.dma_start(ofsum[:], fsum_acc[:])
        nc.sync.dma_start(ogs[:], gs_acc[:])

    if not nc.is_finalized():
        nc.finalize()
    return nc


def _prep_host(inputs):
    f = np.float32
    plasma = np.asarray(inputs["plasma_state"], f)
    W_enc = np.asarray(inputs["W_enc"], f); b_enc = np.asarray(inputs["b_enc"], f)
    g_enc = np.asarray(inputs["g_enc"], f); be_enc = np.asarray(inputs["be_enc"], f)
    W1 = np.asarray(inputs["W1"], f); b1 = np.asarray(inputs["b1"], f)
    g1 = np.asarray(inputs["g1"], f); be1 = np.asarray(inputs["be1"], f)
    W2 = np.asarray(inputs["W2"], f); b2 = np.asarray(inputs["b2"], f)
    g2 = np.asarray(inputs["g2"], f); be2 = np.asarray(inputs["be2"], f)
    Wf = np.asarray(inputs["Wf"], f); bf = np.asarray(inputs["bf"], f)

    we = np.ascontiguousarray(np.vstack([W_enc, b_enc[None, :]]), f)
    w1g = np.ascontiguousarray(g_enc[:, None] * W1, f)
    a1w = np.ascontiguousarray((be_enc @ W1 + b1)[None, :], f)
    w2g = np.ascontiguousarray(g1[:, None] * W2, f)
    a2w = np.ascontiguousarray((be1 @ W2 + b2)[None, :], f)
    wfg = np.ascontiguousarray(g2[:, None] * Wf, f)
    afw = np.ascontiguousarray((be2 @ Wf + bf)[None, :], f)
    ident = np.eye(P, dtype=f).astype(BF16)
    mask = np.zeros((P, NG), f)
    mask[:, ::GRID] = -1e30

    shared = dict(we=we.astype(BF16), w1=w1g.astype(BF16), a1w=a1w.astype(BF16),
                  w2=w2g.astype(BF16), a2w=a2w.astype(BF16), wf=wfg.astype(BF16),
                  afw=afw.astype(BF16), ident=ident, mask=mask)
    in_maps = []
    for c in range(NCORES):
        sh = plasma[c * BS:(c + 1) * BS]
        pTn = np.ascontiguousarray(
            np.vstack([sh.T, np.ones((1, BS), f)])).astype(BF16)
        negs = np.ascontiguousarray(-sh[:, 0:1], f)
        m = dict(shared)
        m["pT"] = pTn
        m["negs"] = negs
        in_maps.append(m)
    return in_maps, plasma


def _get_runner():
    """Build (once) a cached jitted SPMD executable over 8 cores.

    Mirrors concourse.bass2jax.run_bass_via_pjrt's multi-core path but keeps
    the jitted callable so repeat kernel() calls skip re-tracing/compiling.
    """
    if "runner" in _CACHE:
        return _CACHE["runner"]
    import jax
    import concourse.mybir as mybir
    from jax.sharding import Mesh, PartitionSpec
    from jax.experimental.shard_map import shard_map
    from concourse.bass2jax import (_bass_exec_p, install_neuronx_cc_hook,
                                    partition_id_tensor)

    if "nc" not in _CACHE:
        _CACHE["nc"] = _build_bass()
    nc = _CACHE["nc"]
    install_neuronx_cc_hook()

    partition_name = (nc.partition_id_tensor.name
                      if nc.partition_id_tensor else None)
    in_names, out_names, out_avals, zero_outs = [], [], [], []
    for alloc in nc.m.functions[0].allocations:
        if not isinstance(alloc, mybir.MemoryLocationSet):
            continue
        name = alloc.memorylocations[0].name
        if alloc.kind == "ExternalInput":
            if name != partition_name:
                in_names.append(name)
        elif alloc.kind == "ExternalOutput":
            out_names.append(name)
            shape = tuple(alloc.tensor_shape)
            dtype = mybir.dt.np(alloc.dtype)
            out_avals.append(jax.core.ShapedArray(shape, dtype))
            zero_outs.append(np.zeros(shape, dtype))
    n_params = len(in_names)
    n_outs = len(out_avals)
    all_names = in_names + out_names
    if partition_name is not None:
        all_names = all_names + [partition_name]

    def _body(*args):
        operands = list(args)
        if partition_name is not None:
            operands.append(partition_id_tensor())
        outs = _bass_exec_p.bind(
            *operands,
            out_avals=tuple(out_avals),
            in_names=tuple(all_names),
            out_names=tuple(out_names),
            lowering_input_output_aliases=(),
            sim_require_finite=True,
            sim_require_nnan=True,
            nc=nc,
        )
        return tuple(outs)

    devices = jax.devices()[:NCORES]
    mesh = Mesh(np.asarray(devices), ("core",))
    in_specs = (PartitionSpec("core"),) * (n_params + n_outs)
    out_specs = (PartitionSpec("core"),) * n_outs
    donate = tuple(range(n_params, n_params + n_outs))
    sharded = jax.jit(
        shard_map(_body, mesh=mesh, in_specs=in_specs, out_specs=out_specs,
                  check_rep=False),
        donate_argnums=donate, keep_unused=True)

    def run(in_maps):
        concat_in = [
            np.concatenate([np.asarray(in_maps[c][i_name])
                            for c in range(NCORES)], axis=0)
            for i_name in in_names
        ]
        concat_zeros = [
            np.zeros((NCORES * z.shape[0], *z.shape[1:]), z.dtype)
            for z in zero_outs
        ]
        out_arrs = sharded(*concat_in, *concat_zeros)
        return [
            {name: np.asarray(out_arrs[i]).reshape(NCORES, *out_avals[i].shape)[c]
             for i, name in enumerate(out_names)}
            for c in range(NCORES)
        ]

    _CACHE["runner"] = run
    return run


def kernel(**inputs):
    global LAST_RESULTS
    run = _get_runner()
    in_maps, plasma = _prep_host(inputs)
    results = run(in_maps)
    res = type("R", (), {"results": results, "exec_time_ns": None})()
    LAST_RESULTS = res

    f = np.float32
    flux = np.empty((B, GRID, GRID), f)
    fsum = np.empty(B, f)
    gs_rows = np.empty(B, f)
    for c in range(NCORES):
        out = res.results[c]
        flux[c * BS:(c + 1) * BS] = out["oflux"].reshape(BS, GRID, GRID)
        fsum[c * BS:(c + 1) * BS] = np.ascontiguousarray(out["ofsum"].T).reshape(BS)
        gs_rows[c * BS:(c + 1) * BS] = np.ascontiguousarray(out["ogs"].T).reshape(BS)

    s = plasma[:, 0].astype(np.float64)
    n_bound = NG - 62 * 62    # boundary cells: lap = 0 -> (0 - s)^2 each
    gs_residual = (gs_rows.astype(np.float64).sum()
                   + n_bound * np.square(s).sum()) / (B * NG)
    current_consistency = np.mean(np.square(s - fsum.astype(np.float64)))
    pressure = np.exp(f(-2.0) * fsum / f(NG)).astype(f)
    q = np.linspace(0.8, 3.5, GRID).astype(f)
    q_profile = np.broadcast_to(q, (B, GRID)).copy()
    pw = np.asarray(inputs["physics_weights"], f)
    stability = f(np.mean(np.maximum(f(1.1) - q_profile.min(axis=1), f(0.0))))
    physics_loss = f(pw[0] * f(gs_residual) + pw[1] * f(current_consistency)
                     + pw[2] * stability)
    return flux, q_profile, pressure, physics_loss


# revision 28
# speedup vs baseline: 79656.1591x; 79656.1591x over previous
"""Trainium2 Bass kernel for PhysicsInformedMHDSolver.

Data-parallel over 8 NeuronCores: each core runs batch shard of 2048 rows
through MLP (8->256->512->256->4096, gelu+LN via erf + folded affine),
tanh -> segmented cummax (single DVE scan w/ additive reset mask) ->
sigmoid (+row-sum accum) -> 5-pt Laplacian residual accum.
Host combines per-row partials into pressure / physics_loss and builds the
constant q_profile.
"""

import numpy as np
import ml_dtypes

BF16 = ml_dtypes.bfloat16
GRID = 64
NG = GRID * GRID          # 4096
B = 16384
NCORES = 8
BS = B // NCORES          # 2048 per core
P = 128
NT = BS // P              # 16 tiles per core
SQ2INV = 0.7071067811865476
EPS = 1e-5
HID = [256, 512, 256]

_CACHE = {}

# results of the last device run (test.py reads this for profiling)
LAST_RESULTS = None


def _build_bass():
    import concourse.bacc as bacc
    import concourse.mybir as mybir
    import concourse.tile as tile
    from contextlib import ExitStack

    dt = mybir.dt
    f32 = dt.float32
    u32 = dt.uint32
    bf16 = dt.bfloat16
    Alu = mybir.AluOpType
    Act = mybir.ActivationFunctionType

    nc = bacc.Bacc()

    # ---- DRAM I/O ----
    pT = nc.dram_tensor("pT", [9, BS], bf16, kind="ExternalInput")       # plasma^T + ones row
    negs = nc.dram_tensor("negs", [BS, 1], f32, kind="ExternalInput")   # -plasma[:,0]
    we = nc.dram_tensor("we", [9, HID[0]], bf16, kind="ExternalInput")   # [W_enc; b_enc]
    w1 = nc.dram_tensor("w1", [HID[0], HID[1]], bf16, kind="ExternalInput")
    a1w = nc.dram_tensor("a1w", [1, HID[1]], bf16, kind="ExternalInput")  # [bias']
    w2 = nc.dram_tensor("w2", [HID[1], HID[2]], bf16, kind="ExternalInput")
    a2w = nc.dram_tensor("a2w", [1, HID[2]], bf16, kind="ExternalInput")
    wf = nc.dram_tensor("wf", [HID[2], NG], bf16, kind="ExternalInput")
    afw = nc.dram_tensor("afw", [1, NG], bf16, kind="ExternalInput")
    ident = nc.dram_tensor("ident", [P, P], bf16, kind="ExternalInput")
    mask = nc.dram_tensor("mask", [P, NG], f32, kind="ExternalInput")   # scan reset mask
    oflux = nc.dram_tensor("oflux", [BS, NG], f32, kind="ExternalOutput")
    ofsum = nc.dram_tensor("ofsum", [P, NT], f32, kind="ExternalOutput")
    ogs = nc.dram_tensor("ogs", [P, NT], f32, kind="ExternalOutput")

    with ExitStack() as ctx:
        tc = ctx.enter_context(tile.TileContext(nc))
        const = ctx.enter_context(tc.tile_pool(name="const", bufs=1))
        pst = ctx.enter_context(tc.tile_pool(name="pst", bufs=6))
        pwork = ctx.enter_context(tc.tile_pool(name="pwork", bufs=3))
        pbig = ctx.enter_context(tc.tile_pool(name="pbig", bufs=2))
        pfl = ctx.enter_context(tc.tile_pool(name="pfl", bufs=2))
        plap = ctx.enter_context(tc.tile_pool(name="plap", bufs=2))
        ppz = ctx.enter_context(tc.tile_pool(name="ppz", bufs=3, space="PSUM"))
        ppt = ctx.enter_context(tc.tile_pool(name="ppt", bufs=1, space="PSUM"))
        ppf = ctx.enter_context(tc.tile_pool(name="ppf", bufs=2, space="PSUM"))

        # ---- load constants into SBUF ----
        we_sb = const.tile([9, HID[0]], bf16)
        nc.sync.dma_start(we_sb[:], we[:])
        w1_sb = []
        for j in range(2):
            w1j = const.tile([P, HID[1]], bf16, tag=f"w1_{j}", name=f"w1_{j}")
            nc.sync.dma_start(w1j[:], w1[j * P:(j + 1) * P, :])
            w1_sb.append(w1j)
        a1w_sb = const.tile([1, HID[1]], bf16)
        nc.sync.dma_start(a1w_sb[:], a1w[:])
        w2_sb = []
        for j in range(4):
            w2j = const.tile([P, HID[2]], bf16, tag=f"w2_{j}", name=f"w2_{j}")
            nc.sync.dma_start(w2j[:], w2[j * P:(j + 1) * P, :])
            w2_sb.append(w2j)
        a2w_sb = const.tile([1, HID[2]], bf16)
        nc.sync.dma_start(a2w_sb[:], a2w[:])
        wf_sb = []
        for j in range(2):
            wfj = const.tile([P, NG], bf16, tag=f"wf_{j}", name=f"wf_{j}")
            nc.sync.dma_start(wfj[:], wf[j * P:(j + 1) * P, :])
            wf_sb.append(wfj)
        afw_sb = const.tile([1, NG], bf16)
        nc.sync.dma_start(afw_sb[:], afw[:])
        ident_sb = const.tile([P, P], bf16)
        nc.sync.dma_start(ident_sb[:], ident[:])
        mask_sb = const.tile([P, NG], f32)
        nc.sync.dma_start(mask_sb[:], mask[:])

        one_i = const.tile([P, 1], u32)
        nc.vector.memset(one_i[:], 1)
        magic_i = const.tile([P, 1], u32)
        nc.vector.memset(magic_i[:], 0x5F3759DF)

        ones_row = const.tile([1, P], bf16)
        nc.vector.memset(ones_row[:], 1.0)
        fsum_acc = const.tile([P, NT], f32)
        gs_acc = const.tile([P, NT], f32)

        # PE warm-reads of every weight tile: makes the tensor engine's
        # vector clock observe all const-DMA queue semaphores once, so the
        # real matmul groups don't exceed the per-instruction sync-wait cap.
        pe_read = [we_sb, *w1_sb, a1w_sb, *w2_sb, a2w_sb, *wf_sb, afw_sb,
                   ident_sb]
        for i, cts in enumerate(pe_read):
            dz = ppt.tile([1, 1], f32, tag="tp", name=f"dz{i}")
            nc.tensor.matmul(dz[:], lhsT=cts[0:1, 0:1], rhs=cts[0:1, 0:1],
                             start=True, stop=True)

        def gelu_norm(z, n, lname):
            """z: PSUM [P,n] true pre-gelu.  Returns un = LayerNorm(gelu(z))
            (unit affine; g/beta folded into weights host-side).  Uses
            u = (1+erf(z/sqrt2))*z = 2*gelu(z); LN is scale-invariant up to
            the eps term, handled exactly via ve = var(u) + 4*eps."""
            e = pwork.tile([P, n], bf16, tag=f"e{lname}", name=f"e{lname}")
            nc.scalar.activation(e[:], z[:], Act.Erf, bias=0.0, scale=SQ2INV)
            u = pwork.tile([P, n], bf16, tag=f"u{lname}", name=f"u{lname}")
            nc.vector.scalar_tensor_tensor(u[:], e[:], 1.0, z[:], Alu.add, Alu.mult)
            st = pst.tile([P, 6], f32, tag="st", name="st")
            nc.vector.bn_stats(st[:], u[:])
            mv = pst.tile([P, 2], f32, tag="mv", name="mv")
            nc.vector.bn_aggr(mv[:], st[:])
            mu = mv[:, 0:1]
            ve = pst.tile([P, 1], f32, tag="ve", name="ve")
            nc.vector.tensor_scalar_add(ve[:], mv[:, 1:2], 4.0 * EPS)
            # rsqrt via magic-constant + Newton (all DVE; no ACT table swap)
            ish = pst.tile([P, 1], u32, tag="ish", name="ish")
            nc.vector.tensor_tensor(ish[:], ve[:].bitcast(u32), one_i[:],
                                    Alu.logical_shift_right)
            y = pst.tile([P, 1], f32, tag=f"y{lname}", name=f"y{lname}")
            nc.vector.tensor_tensor(y[:].bitcast(u32), magic_i[:], ish[:], Alu.subtract)
            tn = pst.tile([P, 1], f32, tag="tn", name="tn")
            for _ in range(3):
                nc.vector.tensor_mul(tn[:], y[:], y[:])
                nc.vector.tensor_mul(tn[:], tn[:], ve[:])
                nc.vector.tensor_scalar(tn[:], tn[:], -0.5, 1.5, Alu.mult, Alu.add)
                nc.vector.tensor_mul(y[:], y[:], tn[:])
            un = pwork.tile([P, n], bf16, tag=f"un{lname}", name=f"un{lname}")
            nc.vector.tensor_scalar(un[:], u[:], mu, y[:],
                                    Alu.subtract, Alu.mult)
            return un

        def transpose_set(u, nblk, lname, use_pe):
            uT = pwork.tile([P, nblk * P], bf16, tag=f"uT{lname}", name=f"uT{lname}")
            for j in range(nblk):
                if use_pe:
                    tp = ppt.tile([P, P], bf16, tag="tp", name="tp")
                    nc.tensor.transpose(tp[:], u[:, j * P:(j + 1) * P], ident_sb[:])
                    nc.vector.tensor_copy(uT[:, j * P:(j + 1) * P], tp[:])
                else:
                    nc.sync.dma_start(uT[:, j * P:(j + 1) * P],
                                      u[:, j * P:(j + 1) * P], transpose=True)
            return uT

        def do_mlp(t):
            # ---- encoder ----
            xT = pwork.tile([9, P], bf16, tag="xT", name="xT")
            nc.sync.dma_start(xT[:], pT[:, t * P:(t + 1) * P])
            z1 = ppz.tile([P, HID[0]], f32, tag="z", name="z1")
            nc.tensor.matmul(z1[:], lhsT=xT[:], rhs=we_sb[:], start=True, stop=True)
            u1 = gelu_norm(z1, HID[0], "1")

            # ---- layer 1: 256 -> 512 ----
            u1T = transpose_set(u1, 2, "1", use_pe=True)
            z2 = ppz.tile([P, HID[1]], f32, tag="z", name="z2")
            for j in range(2):
                nc.tensor.matmul(z2[:], lhsT=u1T[:, j * P:(j + 1) * P],
                                 rhs=w1_sb[j][:], start=(j == 0), stop=False)
            nc.tensor.matmul(z2[:], lhsT=ones_row[:], rhs=a1w_sb[:], start=False, stop=True)
            u2 = gelu_norm(z2, HID[1], "2")

            # ---- layer 2: 512 -> 256 ----
            u2T = transpose_set(u2, 4, "2", use_pe=True)
            z3 = ppz.tile([P, HID[2]], f32, tag="z", name="z3")
            for j in range(4):
                nc.tensor.matmul(z3[:], lhsT=u2T[:, j * P:(j + 1) * P],
                                 rhs=w2_sb[j][:], start=(j == 0), stop=False)
            nc.tensor.matmul(z3[:], lhsT=ones_row[:], rhs=a2w_sb[:], start=False, stop=True)
            u3 = gelu_norm(z3, HID[2], "3")

            # ---- final layer: 256 -> 4096, tanh ----
            u3T = transpose_set(u3, 2, "3", use_pe=True)
            return u3T

        def stage_b(t, st):
            u3T = st
            fy = pbig.tile([P, NG], f32, tag="fbig", name="fy")
            for q in range(4):
                pf = ppf.tile([P, 1024], f32, tag="pf", name="pf")
                for j in range(2):
                    for s in range(2):
                        off = q * 1024 + s * 512
                        nc.tensor.matmul(pf[:, s * 512:(s + 1) * 512],
                                         lhsT=u3T[:, j * P:(j + 1) * P],
                                         rhs=wf_sb[j][:, off:off + 512],
                                         start=(j == 0), stop=False)
                for s in range(2):
                    off = q * 1024 + s * 512
                    nc.tensor.matmul(pf[:, s * 512:(s + 1) * 512], lhsT=ones_row[:],
                                     rhs=afw_sb[:, off:off + 512],
                                     start=False, stop=True)
                nc.scalar.activation(fy[:, q * 1024:(q + 1) * 1024], pf[:],
                                     Act.Tanh, bias=0.0, scale=1.0)
            # cummax along each 64-wide row: one segmented scan
            fx = pbig.tile([P, NG], f32, tag="fbig", name="fx")
            nc.vector.tensor_tensor_scan(fx[:], mask_sb[:], fy[:], 0.0,
                                         Alu.add, Alu.max)
            return fx

        def stage_c(t, fx):
            fl = pfl.tile([P, NG], f32, tag="fl", name="fl")
            nc.scalar.activation(fl[:], fx[:], Act.Sigmoid, bias=0.0, scale=1.0,
                                 accum_out=fsum_acc[:, t:t + 1])
            nc.sync.dma_start(oflux[t * P:(t + 1) * P, :], fl[:])
            return fl

        def stage_cc(t, fl):
            return fl

        def stage_d(t, fl):
            f3 = fl[:].rearrange("p (r c) -> p r c", r=GRID, c=GRID)
            a1t = plap.tile([P, 62, 62], f32, tag="lapA", name="a1t")
            nc.vector.tensor_add(a1t[:], f3[:, 0:62, 1:63], f3[:, 2:64, 1:63])
            a2t = plap.tile([P, 62, 62], f32, tag="lapB", name="a2t")
            nc.vector.tensor_add(a2t[:], f3[:, 1:63, 0:62], f3[:, 1:63, 2:64])
            nc.vector.scalar_tensor_tensor(a1t[:], f3[:, 1:63, 1:63], -4.0, a1t[:],
                                           Alu.mult, Alu.add)
            nc.vector.tensor_add(a1t[:], a1t[:], a2t[:])
            return a1t, a2t

        def stage_e(t, lap):
            a1t, a2t = lap
            ns = pst.tile([P, 1], f32, tag="ns", name="ns")
            nc.sync.dma_start(ns[:], negs[t * P:(t + 1) * P, :])
            nc.scalar.activation(a2t[:], a1t[:], Act.Square, bias=ns[:], scale=1.0,
                                 accum_out=gs_acc[:, t:t + 1])

        # 5-deep software pipeline: each engine's in-order queue always has
        # ready work; the long scan->sigmoid->lap tail of tile t overlaps the
        # MLP/matmul stages of tiles t+1..t+4.
        sa, sb, sc, scc, sd = {}, {}, {}, {}, {}
        for t in range(NT + 5):
            if t < NT:
                sa[t] = do_mlp(t)
            if 0 <= t - 1 < NT:
                sb[t - 1] = stage_b(t - 1, sa.pop(t - 1))
            if 0 <= t - 2 < NT:
                sc[t - 2] = stage_c(t - 2, sb.pop(t - 2))
            if 0 <= t - 3 < NT:
                scc[t - 3] = stage_cc(t - 3, sc.pop(t - 3))
            if 0 <= t - 4 < NT:
                sd[t - 4] = stage_d(t - 4, scc.pop(t - 4))
            if 0 <= t - 5 < NT:
                stage_e(t - 5, sd.pop(t - 5))

        nc.sync.dma_start(ofsum[:], fsum_acc[:])
        nc.sync.dma_start(ogs[:], gs_acc[:])

    if not nc.is_finalized():
        nc.finalize()
    return nc


def _prep_host(inputs):
    f = np.float32
    plasma = np.asarray(inputs["plasma_state"], f)
    W_enc = np.asarray(inputs["W_enc"], f); b_enc = np.asarray(inputs["b_enc"], f)
    g_enc = np.asarray(inputs["g_enc"], f); be_enc = np.asarray(inputs["be_enc"], f)
    W1 = np.asarray(inputs["W1"], f); b1 = np.asarray(inputs["b1"], f)
    g1 = np.asarray(inputs["g1"], f); be1 = np.asarray(inputs["be1"], f)
    W2 = np.asarray(inputs["W2"], f); b2 = np.asarray(inputs["b2"], f)
    g2 = np.asarray(inputs["g2"], f); be2 = np.asarray(inputs["be2"], f)
    Wf = np.asarray(inputs["Wf"], f); bf = np.asarray(inputs["bf"], f)

    we = np.ascontiguousarray(np.vstack([W_enc, b_enc[None, :]]), f)
    w1g = np.ascontiguousarray(g_enc[:, None] * W1, f)
    a1w = np.ascontiguousarray((be_enc @ W1 + b1)[None, :], f)
    w2g = np.ascontiguousarray(g1[:, None] * W2, f)
    a2w = np.ascontiguousarray((be1 @ W2 + b2)[None, :], f)
    wfg = np.ascontiguousarray(g2[:, None] * Wf, f)
    afw = np.ascontiguousarray((be2 @ Wf + bf)[None, :], f)
    ident = np.eye(P, dtype=f).astype(BF16)
    mask = np.zeros((P, NG), f)
    mask[:, ::GRID] = -1e30

    shared = dict(we=we.astype(BF16), w1=w1g.astype(BF16), a1w=a1w.astype(BF16),
                  w2=w2g.astype(BF16), a2w=a2w.astype(BF16), wf=wfg.astype(BF16),
                  afw=afw.astype(BF16), ident=ident, mask=mask)
    in_maps = []
    for c in range(NCORES):
        sh = plasma[c * BS:(c + 1) * BS]
        pTn = np.ascontiguousarray(
            np.vstack([sh.T, np.ones((1, BS), f)])).astype(BF16)
        negs = np.ascontiguousarray(-sh[:, 0:1], f)
        m = dict(shared)
        m["pT"] = pTn
        m["negs"] = negs
        in_maps.append(m)
    return in_maps, plasma


def _get_runner():
    """Build (once) a cached jitted SPMD executable over 8 cores.

    Mirrors concourse.bass2jax.run_bass_via_pjrt's multi-core path but keeps
    the jitted callable so repeat kernel() calls skip re-tracing/compiling.
    """
    if "runner" in _CACHE:
        return _CACHE["runner"]
    import jax
    import concourse.mybir as mybir
    from jax.sharding import Mesh, PartitionSpec
    from jax.experimental.shard_map import shard_map
    from concourse.bass2jax import (_bass_exec_p, install_neuronx_cc_hook,
                                    partition_id_tensor)

    if "nc" not in _CACHE:
        _CACHE["nc"] = _build_bass()
    nc = _CACHE["nc"]
    install_neuronx_cc_hook()

    partition_name = (nc.partition_id_tensor.name
                      if nc.partition_id_tensor else None)
    in_names, out_names, out_avals, zero_outs = [], [], [], []
    for alloc in nc.m.functions[0].allocations:
        if not isinstance(alloc, mybir.MemoryLocationSet):
            continue
        name = alloc.memorylocations[0].name
        if alloc.kind == "ExternalInput":
            if name != partition_name:
                in_names.append(name)
        elif alloc.kind == "ExternalOutput":
            out_names.append(name)
            shape = tuple(alloc.tensor_shape)
            dtype = mybir.dt.np(alloc.dtype)
            out_avals.append(jax.core.ShapedArray(shape, dtype))
            zero_outs.append(np.zeros(shape, dtype))
    n_params = len(in_names)
    n_outs = len(out_avals)
    all_names = in_names + out_names
    if partition_name is not None:
        all_names = all_names + [partition_name]

    def _body(*args):
        operands = list(args)
        if partition_name is not None:
            operands.append(partition_id_tensor())
        outs = _bass_exec_p.bind(
            *operands,
            out_avals=tuple(out_avals),
            in_names=tuple(all_names),
            out_names=tuple(out_names),
            lowering_input_output_aliases=(),
            sim_require_finite=True,
            sim_require_nnan=True,
            nc=nc,
        )
        return tuple(outs)

    devices = jax.devices()[:NCORES]
    mesh = Mesh(np.asarray(devices), ("core",))
    in_specs = (PartitionSpec("core"),) * (n_params + n_outs)
    out_specs = (PartitionSpec("core"),) * n_outs
    donate = tuple(range(n_params, n_params + n_outs))
    sharded = jax.jit(
        shard_map(_body, mesh=mesh, in_specs=in_specs, out_specs=out_specs,
                  check_rep=False),
        donate_argnums=donate, keep_unused=True)

    def run(in_maps):
        concat_in = [
            np.concatenate([np.asarray(in_maps[c][i_name])
                            for c in range(NCORES)], axis=0)
            for i_name in in_names
        ]
        concat_zeros = [
            np.zeros((NCORES * z.shape[0], *z.shape[1:]), z.dtype)
            for z in zero_outs
        ]
        out_arrs = sharded(*concat_in, *concat_zeros)
        return [
            {name: np.asarray(out_arrs[i]).reshape(NCORES, *out_avals[i].shape)[c]
             for i, name in enumerate(out_names)}
            for c in range(NCORES)
        ]

    _CACHE["runner"] = run
    return run


def kernel(**inputs):
    global LAST_RESULTS
    run = _get_runner()
    in_maps, plasma = _prep_host(inputs)
    results = run(in_maps)
    res = type("R", (), {"results": results, "exec_time_ns": None})()
    LAST_RESULTS = res

    f = np.float32
    flux = np.empty((B, GRID, GRID), f)
    fsum = np.empty(B, f)
    gs_rows = np.empty(B, f)
    for c in range(NCORES):
        out = res.results[c]
        flux[c * BS:(c + 1) * BS] = out["oflux"].reshape(BS, GRID, GRID)
        fsum[c * BS:(c + 1) * BS] = np.ascontiguousarray(out["ofsum"].T).reshape(BS)
        gs_rows[c * BS:(c + 1) * BS] = np.ascontiguousarray(out["ogs"].T).reshape(BS)

    s = plasma[:, 0].astype(np.float64)
    n_bound = NG - 62 * 62    # boundary cells: lap = 0 -> (0 - s)^2 each
    gs_residual = (gs_rows.astype(np.float64).sum()
                   + n_bound * np.square(s).sum()) / (B * NG)
    current_consistency = np.mean(np.square(s - fsum.astype(np.float64)))
    pressure = np.exp(f(-2.0) * fsum / f(NG)).astype(f)
    q = np.linspace(0.8, 3.5, GRID).astype(f)
    q_profile = np.broadcast_to(q, (B, GRID)).copy()
    pw = np.asarray(inputs["physics_weights"], f)
    stability = f(np.mean(np.maximum(f(1.1) - q_profile.min(axis=1), f(0.0))))
    physics_loss = f(pw[0] * f(gs_residual) + pw[1] * f(current_consistency)
                     + pw[2] * stability)
    return flux, q_profile, pressure, physics_loss


# revision 29
# speedup vs baseline: 86835.4566x; 1.0901x over previous
"""Trainium2 Bass kernel for PhysicsInformedMHDSolver.

Data-parallel over 8 NeuronCores: each core runs batch shard of 2048 rows
through MLP (8->256->512->256->4096, gelu+LN via erf + folded affine),
tanh -> segmented cummax (single DVE scan w/ additive reset mask) ->
sigmoid (+row-sum accum) -> 5-pt Laplacian residual accum.
Host combines per-row partials into pressure / physics_loss and builds the
constant q_profile.
"""

import numpy as np
import ml_dtypes

BF16 = ml_dtypes.bfloat16
GRID = 64
NG = GRID * GRID          # 4096
B = 16384
NCORES = 8
BS = B // NCORES          # 2048 per core
P = 128
NT = BS // P              # 16 tiles per core
SQ2INV = 0.7071067811865476
EPS = 1e-5
HID = [256, 512, 256]

_CACHE = {}

# results of the last device run (test.py reads this for profiling)
LAST_RESULTS = None


def _build_bass():
    import concourse.bacc as bacc
    import concourse.mybir as mybir
    import concourse.tile as tile
    from contextlib import ExitStack

    dt = mybir.dt
    f32 = dt.float32
    u32 = dt.uint32
    bf16 = dt.bfloat16
    Alu = mybir.AluOpType
    Act = mybir.ActivationFunctionType

    nc = bacc.Bacc()

    # ---- DRAM I/O ----
    pT = nc.dram_tensor("pT", [9, BS], bf16, kind="ExternalInput")       # plasma^T + ones row
    negs = nc.dram_tensor("negs", [BS, 1], f32, kind="ExternalInput")   # -plasma[:,0]
    we = nc.dram_tensor("we", [9, HID[0]], bf16, kind="ExternalInput")   # [W_enc; b_enc]
    w1 = nc.dram_tensor("w1", [HID[0], HID[1]], bf16, kind="ExternalInput")
    a1w = nc.dram_tensor("a1w", [1, HID[1]], bf16, kind="ExternalInput")  # [bias']
    w2 = nc.dram_tensor("w2", [HID[1], HID[2]], bf16, kind="ExternalInput")
    a2w = nc.dram_tensor("a2w", [1, HID[2]], bf16, kind="ExternalInput")
    wf = nc.dram_tensor("wf", [HID[2], NG], bf16, kind="ExternalInput")
    afw = nc.dram_tensor("afw", [1, NG], bf16, kind="ExternalInput")
    ident = nc.dram_tensor("ident", [P, P], bf16, kind="ExternalInput")
    mask = nc.dram_tensor("mask", [P, NG], f32, kind="ExternalInput")   # scan reset mask
    oflux = nc.dram_tensor("oflux", [BS, NG], f32, kind="ExternalOutput")
    ofsum = nc.dram_tensor("ofsum", [P, NT], f32, kind="ExternalOutput")
    ogs = nc.dram_tensor("ogs", [P, NT], f32, kind="ExternalOutput")

    with ExitStack() as ctx:
        tc = ctx.enter_context(tile.TileContext(nc))
        const = ctx.enter_context(tc.tile_pool(name="const", bufs=1))
        pst = ctx.enter_context(tc.tile_pool(name="pst", bufs=6))
        pwork = ctx.enter_context(tc.tile_pool(name="pwork", bufs=3))
        pbig = ctx.enter_context(tc.tile_pool(name="pbig", bufs=3))
        pfl = ctx.enter_context(tc.tile_pool(name="pfl", bufs=2))
        plap = ctx.enter_context(tc.tile_pool(name="plap", bufs=2))
        ppz = ctx.enter_context(tc.tile_pool(name="ppz", bufs=3, space="PSUM"))
        ppt = ctx.enter_context(tc.tile_pool(name="ppt", bufs=1, space="PSUM"))
        ppf = ctx.enter_context(tc.tile_pool(name="ppf", bufs=2, space="PSUM"))

        # ---- load constants into SBUF ----
        we_sb = const.tile([9, HID[0]], bf16)
        nc.sync.dma_start(we_sb[:], we[:])
        w1_sb = []
        for j in range(2):
            w1j = const.tile([P, HID[1]], bf16, tag=f"w1_{j}", name=f"w1_{j}")
            nc.sync.dma_start(w1j[:], w1[j * P:(j + 1) * P, :])
            w1_sb.append(w1j)
        a1w_sb = const.tile([1, HID[1]], bf16)
        nc.sync.dma_start(a1w_sb[:], a1w[:])
        w2_sb = []
        for j in range(4):
            w2j = const.tile([P, HID[2]], bf16, tag=f"w2_{j}", name=f"w2_{j}")
            nc.sync.dma_start(w2j[:], w2[j * P:(j + 1) * P, :])
            w2_sb.append(w2j)
        a2w_sb = const.tile([1, HID[2]], bf16)
        nc.sync.dma_start(a2w_sb[:], a2w[:])
        wf_sb = []
        for j in range(2):
            wfj = const.tile([P, NG], bf16, tag=f"wf_{j}", name=f"wf_{j}")
            nc.sync.dma_start(wfj[:], wf[j * P:(j + 1) * P, :])
            wf_sb.append(wfj)
        afw_sb = const.tile([1, NG], bf16)
        nc.sync.dma_start(afw_sb[:], afw[:])
        ident_sb = const.tile([P, P], bf16)
        nc.sync.dma_start(ident_sb[:], ident[:])
        mask_sb = const.tile([P, NG], f32)
        nc.sync.dma_start(mask_sb[:], mask[:])

        one_i = const.tile([P, 1], u32)
        nc.vector.memset(one_i[:], 1)
        magic_i = const.tile([P, 1], u32)
        nc.vector.memset(magic_i[:], 0x5F3759DF)

        ones_row = const.tile([1, P], bf16)
        nc.vector.memset(ones_row[:], 1.0)
        fsum_acc = const.tile([P, NT], f32)
        gs_acc = const.tile([P, NT], f32)

        # PE warm-reads of every weight tile: makes the tensor engine's
        # vector clock observe all const-DMA queue semaphores once, so the
        # real matmul groups don't exceed the per-instruction sync-wait cap.
        pe_read = [we_sb, *w1_sb, a1w_sb, *w2_sb, a2w_sb, *wf_sb, afw_sb,
                   ident_sb]
        for i, cts in enumerate(pe_read):
            dz = ppt.tile([1, 1], f32, tag="tp", name=f"dz{i}")
            nc.tensor.matmul(dz[:], lhsT=cts[0:1, 0:1], rhs=cts[0:1, 0:1],
                             start=True, stop=True)

        def gelu_norm(z, n, lname):
            """z: PSUM [P,n] true pre-gelu.  Returns un = LayerNorm(gelu(z))
            (unit affine; g/beta folded into weights host-side).  Uses
            u = (1+erf(z/sqrt2))*z = 2*gelu(z); LN is scale-invariant up to
            the eps term, handled exactly via ve = var(u) + 4*eps."""
            e = pwork.tile([P, n], bf16, tag=f"e{lname}", name=f"e{lname}")
            nc.scalar.activation(e[:], z[:], Act.Erf, bias=0.0, scale=SQ2INV)
            u = pwork.tile([P, n], bf16, tag=f"u{lname}", name=f"u{lname}")
            nc.vector.scalar_tensor_tensor(u[:], e[:], 1.0, z[:], Alu.add, Alu.mult)
            st = pst.tile([P, 6], f32, tag="st", name="st")
            nc.vector.bn_stats(st[:], u[:])
            mv = pst.tile([P, 2], f32, tag="mv", name="mv")
            nc.vector.bn_aggr(mv[:], st[:])
            mu = mv[:, 0:1]
            ve = pst.tile([P, 1], f32, tag="ve", name="ve")
            nc.vector.tensor_scalar_add(ve[:], mv[:, 1:2], 4.0 * EPS)
            # rsqrt via magic-constant + Newton (all DVE; no ACT table swap)
            ish = pst.tile([P, 1], u32, tag="ish", name="ish")
            nc.vector.tensor_tensor(ish[:], ve[:].bitcast(u32), one_i[:],
                                    Alu.logical_shift_right)
            y = pst.tile([P, 1], f32, tag=f"y{lname}", name=f"y{lname}")
            nc.vector.tensor_tensor(y[:].bitcast(u32), magic_i[:], ish[:], Alu.subtract)
            tn = pst.tile([P, 1], f32, tag="tn", name="tn")
            for _ in range(3):
                nc.vector.tensor_mul(tn[:], y[:], y[:])
                nc.vector.tensor_mul(tn[:], tn[:], ve[:])
                nc.vector.tensor_scalar(tn[:], tn[:], -0.5, 1.5, Alu.mult, Alu.add)
                nc.vector.tensor_mul(y[:], y[:], tn[:])
            un = pwork.tile([P, n], bf16, tag=f"un{lname}", name=f"un{lname}")
            nc.vector.tensor_scalar(un[:], u[:], mu, y[:],
                                    Alu.subtract, Alu.mult)
            return un

        def transpose_set(u, nblk, lname, use_pe):
            uT = pwork.tile([P, nblk * P], bf16, tag=f"uT{lname}", name=f"uT{lname}")
            for j in range(nblk):
                if use_pe:
                    tp = ppt.tile([P, P], bf16, tag="tp", name="tp")
                    nc.tensor.transpose(tp[:], u[:, j * P:(j + 1) * P], ident_sb[:])
                    nc.vector.tensor_copy(uT[:, j * P:(j + 1) * P], tp[:])
                else:
                    nc.sync.dma_start(uT[:, j * P:(j + 1) * P],
                                      u[:, j * P:(j + 1) * P], transpose=True)
            return uT

        def do_mlp(t):
            # ---- encoder ----
            xT = pwork.tile([9, P], bf16, tag="xT", name="xT")
            nc.sync.dma_start(xT[:], pT[:, t * P:(t + 1) * P])
            z1 = ppz.tile([P, HID[0]], f32, tag="z", name="z1")
            nc.tensor.matmul(z1[:], lhsT=xT[:], rhs=we_sb[:], start=True, stop=True)
            u1 = gelu_norm(z1, HID[0], "1")

            # ---- layer 1: 256 -> 512 ----
            u1T = transpose_set(u1, 2, "1", use_pe=True)
            z2 = ppz.tile([P, HID[1]], f32, tag="z", name="z2")
            for j in range(2):
                nc.tensor.matmul(z2[:], lhsT=u1T[:, j * P:(j + 1) * P],
                                 rhs=w1_sb[j][:], start=(j == 0), stop=False)
            nc.tensor.matmul(z2[:], lhsT=ones_row[:], rhs=a1w_sb[:], start=False, stop=True)
            u2 = gelu_norm(z2, HID[1], "2")

            # ---- layer 2: 512 -> 256 ----
            u2T = transpose_set(u2, 4, "2", use_pe=True)
            z3 = ppz.tile([P, HID[2]], f32, tag="z", name="z3")
            for j in range(4):
                nc.tensor.matmul(z3[:], lhsT=u2T[:, j * P:(j + 1) * P],
                                 rhs=w2_sb[j][:], start=(j == 0), stop=False)
            nc.tensor.matmul(z3[:], lhsT=ones_row[:], rhs=a2w_sb[:], start=False, stop=True)
            u3 = gelu_norm(z3, HID[2], "3")

            # ---- final layer: 256 -> 4096, tanh ----
            u3T = transpose_set(u3, 2, "3", use_pe=True)
            return u3T

        def stage_b(t, st):
            u3T = st
            fy = pbig.tile([P, NG], f32, tag="fbig", name="fy")
            for q in range(4):
                pf = ppf.tile([P, 1024], f32, tag="pf", name="pf")
                for j in range(2):
                    for s in range(2):
                        off = q * 1024 + s * 512
                        nc.tensor.matmul(pf[:, s * 512:(s + 1) * 512],
                                         lhsT=u3T[:, j * P:(j + 1) * P],
                                         rhs=wf_sb[j][:, off:off + 512],
                                         start=(j == 0), stop=False)
                for s in range(2):
                    off = q * 1024 + s * 512
                    nc.tensor.matmul(pf[:, s * 512:(s + 1) * 512], lhsT=ones_row[:],
                                     rhs=afw_sb[:, off:off + 512],
                                     start=False, stop=True)
                nc.scalar.activation(fy[:, q * 1024:(q + 1) * 1024], pf[:],
                                     Act.Tanh, bias=0.0, scale=1.0)
            # cummax along each 64-wide row: one segmented scan
            fx = pbig.tile([P, NG], f32, tag="fbig", name="fx")
            nc.vector.tensor_tensor_scan(fx[:], mask_sb[:], fy[:], 0.0,
                                         Alu.add, Alu.max)
            return fx

        def stage_c(t, fx):
            fl = pfl.tile([P, NG], f32, tag="fl", name="fl")
            nc.scalar.activation(fl[:], fx[:], Act.Sigmoid, bias=0.0, scale=1.0,
                                 accum_out=fsum_acc[:, t:t + 1])
            nc.sync.dma_start(oflux[t * P:(t + 1) * P, :], fl[:])
            return fl

        def stage_cc(t, fl):
            # bf16 shadow of flux so the Laplacian adds run in DVE 2x mode
            fb = plap.tile([P, NG], bf16, tag="flbf", name="fb")
            nc.vector.tensor_copy(fb[:], fl[:])
            return fb

        def stage_d(t, fb):
            f3 = fb[:].rearrange("p (r c) -> p r c", r=GRID, c=GRID)
            # vertical part over full columns keeps reads 4B-aligned (2x mode)
            a1t = plap.tile([P, 62, GRID], bf16, tag="lapA", name="a1t")
            nc.vector.tensor_add(a1t[:], f3[:, 0:62, :], f3[:, 2:64, :])
            a2t = plap.tile([P, 62, 62], bf16, tag="lapB", name="a2t")
            nc.vector.tensor_add(a2t[:], f3[:, 1:63, 0:62], f3[:, 1:63, 2:64])
            nc.vector.scalar_tensor_tensor(a1t[:], f3[:, 1:63, :], -4.0, a1t[:],
                                           Alu.mult, Alu.add)
            nc.vector.tensor_add(a2t[:], a1t[:, :, 1:63], a2t[:])
            return a1t, a2t

        def stage_e(t, lap):
            a1t, a2t = lap
            ns = pst.tile([P, 1], f32, tag="ns", name="ns")
            nc.sync.dma_start(ns[:], negs[t * P:(t + 1) * P, :])
            nc.scalar.activation(a1t[:, :, 1:63], a2t[:], Act.Square, bias=ns[:],
                                 scale=1.0, accum_out=gs_acc[:, t:t + 1])

        # 5-deep software pipeline: each engine's in-order queue always has
        # ready work; the long scan->sigmoid->lap tail of tile t overlaps the
        # MLP/matmul stages of tiles t+1..t+4.
        sa, sb, sc, scc, sd = {}, {}, {}, {}, {}
        for t in range(NT + 5):
            if t < NT:
                sa[t] = do_mlp(t)
            if 0 <= t - 1 < NT:
                sb[t - 1] = stage_b(t - 1, sa.pop(t - 1))
            if 0 <= t - 2 < NT:
                sc[t - 2] = stage_c(t - 2, sb.pop(t - 2))
            if 0 <= t - 3 < NT:
                scc[t - 3] = stage_cc(t - 3, sc.pop(t - 3))
            if 0 <= t - 4 < NT:
                sd[t - 4] = stage_d(t - 4, scc.pop(t - 4))
            if 0 <= t - 5 < NT:
                stage_e(t - 5, sd.pop(t - 5))

        nc.sync.dma_start(ofsum[:], fsum_acc[:])
        nc.sync.dma_start(ogs[:], gs_acc[:])

    if not nc.is_finalized():
        nc.finalize()
    return nc


def _prep_host(inputs):
    f = np.float32
    plasma = np.asarray(inputs["plasma_state"], f)
    W_enc = np.asarray(inputs["W_enc"], f); b_enc = np.asarray(inputs["b_enc"], f)
    g_enc = np.asarray(inputs["g_enc"], f); be_enc = np.asarray(inputs["be_enc"], f)
    W1 = np.asarray(inputs["W1"], f); b1 = np.asarray(inputs["b1"], f)
    g1 = np.asarray(inputs["g1"], f); be1 = np.asarray(inputs["be1"], f)
    W2 = np.asarray(inputs["W2"], f); b2 = np.asarray(inputs["b2"], f)
    g2 = np.asarray(inputs["g2"], f); be2 = np.asarray(inputs["be2"], f)
    Wf = np.asarray(inputs["Wf"], f); bf = np.asarray(inputs["bf"], f)

    we = np.ascontiguousarray(np.vstack([W_enc, b_enc[None, :]]), f)
    w1g = np.ascontiguousarray(g_enc[:, None] * W1, f)
    a1w = np.ascontiguousarray((be_enc @ W1 + b1)[None, :], f)
    w2g = np.ascontiguousarray(g1[:, None] * W2, f)
    a2w = np.ascontiguousarray((be1 @ W2 + b2)[None, :], f)
    wfg = np.ascontiguousarray(g2[:, None] * Wf, f)
    afw = np.ascontiguousarray((be2 @ Wf + bf)[None, :], f)
    ident = np.eye(P, dtype=f).astype(BF16)
    mask = np.zeros((P, NG), f)
    mask[:, ::GRID] = -1e30

    shared = dict(we=we.astype(BF16), w1=w1g.astype(BF16), a1w=a1w.astype(BF16),
                  w2=w2g.astype(BF16), a2w=a2w.astype(BF16), wf=wfg.astype(BF16),
                  afw=afw.astype(BF16), ident=ident, mask=mask)
    in_maps = []
    for c in range(NCORES):
        sh = plasma[c * BS:(c + 1) * BS]
        pTn = np.ascontiguousarray(
            np.vstack([sh.T, np.ones((1, BS), f)])).astype(BF16)
        negs = np.ascontiguousarray(-sh[:, 0:1], f)
        m = dict(shared)
        m["pT"] = pTn
        m["negs"] = negs
        in_maps.append(m)
    return in_maps, plasma


def _get_runner():
    """Build (once) a cached jitted SPMD executable over 8 cores.

    Mirrors concourse.bass2jax.run_bass_via_pjrt's multi-core path but keeps
    the jitted callable so repeat kernel() calls skip re-tracing/compiling.
    """
    if "runner" in _CACHE:
        return _CACHE["runner"]
    import jax
    import concourse.mybir as mybir
    from jax.sharding import Mesh, PartitionSpec
    from jax.experimental.shard_map import shard_map
    from concourse.bass2jax import (_bass_exec_p, install_neuronx_cc_hook,
                                    partition_id_tensor)

    if "nc" not in _CACHE:
        _CACHE["nc"] = _build_bass()
    nc = _CACHE["nc"]
    install_neuronx_cc_hook()

    partition_name = (nc.partition_id_tensor.name
                      if nc.partition_id_tensor else None)
    in_names, out_names, out_avals, zero_outs = [], [], [], []
    for alloc in nc.m.functions[0].allocations:
        if not isinstance(alloc, mybir.MemoryLocationSet):
            continue
        name = alloc.memorylocations[0].name
        if alloc.kind == "ExternalInput":
            if name != partition_name:
                in_names.append(name)
        elif alloc.kind == "ExternalOutput":
            out_names.append(name)
            shape = tuple(alloc.tensor_shape)
            dtype = mybir.dt.np(alloc.dtype)
            out_avals.append(jax.core.ShapedArray(shape, dtype))
            zero_outs.append(np.zeros(shape, dtype))
    n_params = len(in_names)
    n_outs = len(out_avals)
    all_names = in_names + out_names
    if partition_name is not None:
        all_names = all_names + [partition_name]

    def _body(*args):
        operands = list(args)
        if partition_name is not None:
            operands.append(partition_id_tensor())
        outs = _bass_exec_p.bind(
            *operands,
            out_avals=tuple(out_avals),
            in_names=tuple(all_names),
            out_names=tuple(out_names),
            lowering_input_output_aliases=(),
            sim_require_finite=True,
            sim_require_nnan=True,
            nc=nc,
        )
        return tuple(outs)

    devices = jax.devices()[:NCORES]
    mesh = Mesh(np.asarray(devices), ("core",))
    in_specs = (PartitionSpec("core"),) * (n_params + n_outs)
    out_specs = (PartitionSpec("core"),) * n_outs
    donate = tuple(range(n_params, n_params + n_outs))
    sharded = jax.jit(
        shard_map(_body, mesh=mesh, in_specs=in_specs, out_specs=out_specs,
                  check_rep=False),
        donate_argnums=donate, keep_unused=True)

    def run(in_maps):
        concat_in = [
            np.concatenate([np.asarray(in_maps[c][i_name])
                            for c in range(NCORES)], axis=0)
            for i_name in in_names
        ]
        concat_zeros = [
            np.zeros((NCORES * z.shape[0], *z.shape[1:]), z.dtype)
            for z in zero_outs
        ]
        out_arrs = sharded(*concat_in, *concat_zeros)
        return [
            {name: np.asarray(out_arrs[i]).reshape(NCORES, *out_avals[i].shape)[c]
             for i, name in enumerate(out_names)}
            for c in range(NCORES)
        ]

    _CACHE["runner"] = run
    return run


def kernel(**inputs):
    global LAST_RESULTS
    run = _get_runner()
    in_maps, plasma = _prep_host(inputs)
    results = run(in_maps)
    res = type("R", (), {"results": results, "exec_time_ns": None})()
    LAST_RESULTS = res

    f = np.float32
    flux = np.empty((B, GRID, GRID), f)
    fsum = np.empty(B, f)
    gs_rows = np.empty(B, f)
    for c in range(NCORES):
        out = res.results[c]
        flux[c * BS:(c + 1) * BS] = out["oflux"].reshape(BS, GRID, GRID)
        fsum[c * BS:(c + 1) * BS] = np.ascontiguousarray(out["ofsum"].T).reshape(BS)
        gs_rows[c * BS:(c + 1) * BS] = np.ascontiguousarray(out["ogs"].T).reshape(BS)

    s = plasma[:, 0].astype(np.float64)
    n_bound = NG - 62 * 62    # boundary cells: lap = 0 -> (0 - s)^2 each
    gs_residual = (gs_rows.astype(np.float64).sum()
                   + n_bound * np.square(s).sum()) / (B * NG)
    current_consistency = np.mean(np.square(s - fsum.astype(np.float64)))
    pressure = np.exp(f(-2.0) * fsum / f(NG)).astype(f)
    q = np.linspace(0.8, 3.5, GRID).astype(f)
    q_profile = np.broadcast_to(q, (B, GRID)).copy()
    pw = np.asarray(inputs["physics_weights"], f)
    stability = f(np.mean(np.maximum(f(1.1) - q_profile.min(axis=1), f(0.0))))
    physics_loss = f(pw[0] * f(gs_residual) + pw[1] * f(current_consistency)
                     + pw[2] * stability)
    return flux, q_profile, pressure, physics_loss
